# revision 63
# baseline (speedup 1.0000x reference)
"""GCN layer (symmetric-normalized message passing + skip) on 8 Trainium2
NeuronCores via Bass/Tile.

    deg = bincount(src); dis = (deg>0) * rsqrt(max(deg,1))
    out = segsum_dst( dis_src*dis_dst * feats[src] ) @ Wm.T + bm
          + feats @ Ws.T + bs

v4 design: STATIC banded scatter masks + minimal gather rows. The whole
kernel is paced by SWDGE dma_gather descriptor generation, which is serial
across queues (each extended instruction barriers all 8 Q7 cores), at
~2.5-5 ns per gathered row. So the layout minimizes gathered rows:

- Nodes sorted globally by in-degree, dealt round-robin to the 8 cores
  (every core sees an identical degree profile; SPMD shares one program).
- Per 128-dst sub-block, every dst gets exactly K = max in-degree in the
  block slots; edge j of dst d sits at slot d*K+j. Pad slots carry w=0.
  Rows gathered: 81664/core (2% over the 80000 edges).
- The scatter matrix for a 128-slot tile is a static 0/1 band depending
  only on (K, 128t mod K): a [128, W] strip (W = (c+127)//K+1 <= 128/K+2)
  used as a narrow matmul rhs accumulating into the PSUM bank at a column
  offset. PSUM banks are pre-zeroed by a k=1 matmul; band matmuls all
  accumulate. ~82 strips, ~2.7KB/partition total.
- Per-tile vector work is one [128,128] tensor_scalar multiplying the
  gathered messages by the per-slot weight w = dis[src]*dis[dst]
  (computed on device from host-supplied integer degrees; ACT Rsqrt).
- Gathers: per-(core, superblock%4) compacted source tables (<32768 rows,
  int16 indices); ops of <=8 tiles (ring capacity), queues round-robin
  (parallelizes the DMA transfers), trailing pad indices are -1 (the Q7
  ucode trims them for free). msgs is quadruple-buffered; flushes are
  emitted before the next superblock's tiles so they fill gather gaps.

Host prep supplies integer partitioning metadata only (permutations, table
row ids, per-slot degrees); all float math runs on device.
"""

import numpy as np

P = 128
D = 128
NCORES = 8
N = 100000
E = 640000
SUBW = 128                  # dst sub-block width (shared K)
SBW = 512                   # superblock width (flush/psum granularity)
NSB = 25                    # superblocks per core
NBLK = NSB * 4              # sub-blocks per core
NLOC = N // NCORES          # 12500
NLOC_PAD = NSB * SBW        # 12800
NPHASE = 4                  # superblock % 4 -> compacted source table
MAXIDX = 32768              # int16 gather index limit
NBUF = 5                    # msgs buffers in flight


# ---------------------------------------------------------------- host prep

def _prep(feats, src, dst, wm, bm, ws, bs):
    import os
    n, d = feats.shape
    assert n == N and d == D
    src = np.asarray(src).astype(np.int64)
    dst = np.asarray(dst).astype(np.int64)
    feats = np.asarray(feats, dtype=np.float32)

    deg = np.bincount(src, minlength=n)            # out-degree -> dis
    keep = deg[dst] > 0                            # dis[dst]=0 edges are 0
    src_k, dst_k = src[keep], dst[keep]
    deg_in = np.bincount(dst_k, minlength=n)

    order = np.argsort(-deg_in, kind="stable")     # rank -> node
    rank_of = np.empty(n, np.int64)
    rank_of[order] = np.arange(n)
    owner = rank_of % NCORES
    lrank = rank_of // NCORES

    Ks = np.empty(NBLK, np.int64)
    for j in range(NBLK):
        band = deg_in[order[j * SUBW * NCORES:(j + 1) * SUBW * NCORES]]
        Ks[j] = max(1, int(band.max())) if len(band) else 1
    sub_base = np.concatenate([[0], np.cumsum(SUBW * Ks)]).astype(np.int64)
    TOT = int(sub_base[-1])
    TILES = TOT // P

    # banded mask strips, one per distinct (K, c); strip[p] = (c+p)//K
    mask_cols = {}
    mcol = 0
    tile_info = []                                 # per tile: (mc, W, bankcol)
    for j in range(NBLK):
        K = int(Ks[j])
        for t in range(K):
            c = (P * t) % K
            d0 = (P * t) // K
            if (K, c) not in mask_cols:
                W = (c + P - 1) // K + 1
                mask_cols[(K, c)] = (mcol, W)
                mcol += W
            mc, W = mask_cols[(K, c)]
            tile_info.append((mc, W, (j % 4) * SUBW + d0))
    MASKC = mcol
    masks = np.zeros((P, MASKC), np.float16)
    pp = np.arange(P)
    for (K, c), (mc, W) in mask_cols.items():
        masks[pp, mc + (c + pp) // K] = 1.0

    sb_tiles = [int(Ks[4 * s:4 * s + 4].sum()) for s in range(NSB)]
    sb_tile_base = [int(sub_base[4 * s]) // P for s in range(NSB)]

    # gather chunking: ops of <=TPO tiles (TPO*8+2 descs/engine must fit the
    # per-queue SWDGE ring; 8 -> 66 fits, 11 -> 90 does not)
    # processing order of superblocks (largest-first, the degree-sort order;
    # smallest-first hung on hardware and was reverted)
    proc_order = list(range(NSB))

    TPO = int(os.environ.get("GCN_TPO", "8"))
    chunks = []                                    # (sb, tile_lo, tile_hi, q)
    qload = [0, 0, 0, 0]                           # rows per queue (balance)
    for s in proc_order:
        t0, tsb = sb_tile_base[s], sb_tiles[s]
        for t in range(0, tsb, TPO):
            lo, hi = t0 + t, t0 + min(tsb, t + TPO)
            q = min(range(4), key=lambda i: qload[i])
            qload[q] += (hi - lo) * P
            chunks.append((s, lo, hi, q))
    chunk_cols = []
    cb = 0
    for (_, lo, hi, _) in chunks:
        ncol = (hi - lo) * P // 16
        chunk_cols.append((cb, ncol))
        cb += ncol
    GIDXC = cb

    # per-core tables and metadata
    per_core = []
    for k in range(NCORES):
        m = owner[dst_k] == k
        s_e, d_e = src_k[m], dst_k[m]
        lr = lrank[d_e]
        o2 = np.argsort(lr, kind="stable")
        s_e, lr = s_e[o2], lr[o2]
        j_e = lr // SUBW
        dloc = lr % SUBW
        cnt = np.arange(len(lr)) - np.searchsorted(lr, lr)
        slot = sub_base[j_e] + dloc * Ks[j_e] + cnt

        phase_e = (j_e // 4) % NPHASE
        tabs = []
        gidx_flat = np.zeros(TOT, np.int64)
        for ph in range(NPHASE):
            mm2 = phase_e == ph
            uniq = np.unique(s_e[mm2])
            assert len(uniq) < MAXIDX, len(uniq)
            r = np.zeros(n, np.int64)
            r[uniq] = np.arange(len(uniq))
            tabs.append(feats[uniq].astype(np.float16))
            gidx_flat[slot[mm2]] = r[s_e[mm2]]

        filled = np.zeros(TOT, bool)
        filled[slot] = True
        # integer degree product; w = rsqrt(prod) masked by prod>0 on device
        prod = np.zeros(TOT, np.float32)
        prod[slot] = (deg[s_e] * deg[dst_k[m][o2]]).astype(np.float32)

        # trailing pad slots -> idx -1 (Q7 trims them); msgs buffers are
        # memset once at startup so trimmed slots stay finite (w=0 then
        # zeroes them in the scatter matmul)
        for (sbi, lo, hi, _), (cb0, ncol) in zip(chunks, chunk_cols):
            f = filled[lo * P:hi * P]
            nz = np.flatnonzero(f)
            tail = (nz[-1] + 1) if len(nz) else 0
            gidx_flat[lo * P + tail:hi * P] = -1

        gidx = np.zeros((P, GIDXC), np.int16)
        for (sbi, lo, hi, _), (cb0, ncol) in zip(chunks, chunk_cols):
            fl = gidx_flat[lo * P:hi * P].astype(np.int16)
            a = fl.reshape(ncol, 16).T
            gidx[:, cb0:cb0 + ncol] = np.tile(a, (8, 1))

        gprod = np.ascontiguousarray(prod.reshape(TILES, P).T)

        ft = np.zeros((P, NLOC_PAD), np.float16)
        rr = np.arange(NLOC)
        ft[:, :NLOC] = feats[order[rr * NCORES + k]].T.astype(np.float16)

        per_core.append((tabs, gidx, gprod, ft))

    TABR = max(t.shape[0] for (tabs, *_r) in per_core for t in tabs)
    TABR = (TABR + 127) // 128 * 128

    wmT = np.ascontiguousarray(np.asarray(wm, np.float32).T).astype(np.float16)
    wsT = np.ascontiguousarray(np.asarray(ws, np.float32).T).astype(np.float16)
    bm = np.asarray(bm, np.float32).reshape(1, D)
    bs = np.asarray(bs, np.float32).reshape(1, D)

    in_maps = []
    for k in range(NCORES):
        tabs, gidx, gprod, ft = per_core[k]
        im = {"gidx": gidx, "gprod": gprod,
              "featsT": ft, "masks": masks,
              "wmT": wmT, "wsT": wsT, "bm": bm, "bs": bs}
        for ph in range(NPHASE):
            tt = np.zeros((TABR, D), np.float16)
            tt[:tabs[ph].shape[0]] = tabs[ph]
            im[f"tab{ph}"] = tt
        in_maps.append(im)

    # per-tile slice into the per-superblock weighted-strip buffer
    wst_cols = []                                  # per sb: total strip cols
    tile_wc = []                                   # per tile: col offset
    for s in range(NSB):
        wc = 0
        for t in range(sb_tiles[s]):
            T0 = sb_tile_base[s] + t
            tile_wc.append(wc)
            wc += tile_info[T0][1]
        wst_cols.append(wc)

    HEADC = 0
    for (s, lo, hi, q), (cb0, ncol) in zip(chunks, chunk_cols):
        if s in (proc_order[0], proc_order[1]):
            HEADC = cb0 + ncol
        else:
            break
    geom = {
        "TOT": TOT, "TILES": TILES, "MASKC": MASKC,
        "tile_info": tile_info, "sb_tiles": sb_tiles,
        "sb_tile_base": sb_tile_base, "chunks": chunks,
        "chunk_cols": chunk_cols, "GIDXC": GIDXC, "TABR": TABR,
        "HEADC": HEADC, "wst_cols": wst_cols, "tile_wc": tile_wc,
        "proc_order": proc_order,
    }
    return in_maps, geom, order


# ------------------------------------------------------------- device kernel

def device_kernel(tc, outs, ins, geom, cfg):
    import concourse.mybir as mybir

    nc = tc.nc
    f32 = mybir.dt.float32
    f16 = mybir.dt.float16
    i16 = mybir.dt.int16
    Op = mybir.AluOpType
    Act = mybir.ActivationFunctionType

    (out_d,) = outs
    (gidx_d, gprod_d, featsT_d, masks_d,
     wmT_d, wsT_d, bm_d, bs_d, tab_ds) = ins

    TILES = geom["TILES"]
    MASKC = geom["MASKC"]
    GIDXC = geom["GIDXC"]
    tile_info = geom["tile_info"]
    sb_tiles = geom["sb_tiles"]
    sb_tile_base = geom["sb_tile_base"]
    chunks = geom["chunks"]
    chunk_cols = geom["chunk_cols"]
    wst_cols = geom["wst_cols"]
    tile_wc = geom["tile_wc"]
    proc_order = geom["proc_order"]

    bufmax = [max(sb_tiles[s] for s in proc_order[i::NBUF])
              for i in range(NBUF)]
    abl = cfg.get("ABL", ())

    with (
        tc.tile_pool(name="sbuf", bufs=1) as sb,
        tc.tile_pool(name="srst", bufs=2) as srst,
        tc.tile_pool(name="sstg", bufs=2) as sstg,
        tc.tile_pool(name="psag", bufs=2, space="PSUM") as psag,
        tc.tile_pool(name="pslin", bufs=2, space="PSUM") as pslin,
    ):
        # ---------------- setup ----------------
        # gidx split: a small head (superblocks 0-1) lands fast so the first
        # gathers don't wait on the full index load (whole-tile deps)
        HEADC = geom["HEADC"]
        gidxh = sb.tile([P, HEADC], i16)
        nc.sync.dma_start(out=gidxh[:], in_=gidx_d[:, :HEADC])
        gidxt = sb.tile([P, GIDXC - HEADC], i16)
        nc.sync.dma_start(out=gidxt[:], in_=gidx_d[:, HEADC:])
        prod = sb.tile([P, TILES], f32, tag="prod")
        nc.sync.dma_start(out=prod[:], in_=gprod_d[:])
        bmt = sb.tile([1, D], f32)
        nc.sync.dma_start(out=bmt[:], in_=bm_d[:])
        bst = sb.tile([1, D], f32)
        nc.sync.dma_start(out=bst[:], in_=bs_d[:])

        msgs_bufs = []
        for i in range(NBUF):
            mt = sb.tile([P, bufmax[i] * P], f16, tag=f"msgs{i}", name=f"m{i}")
            nc.vector.memset(mt[:], 0.0)   # trimmed pad slots stay finite
            msgs_bufs.append(mt)

        sb_chunks = {}
        for (sbi, lo, hi, q), (cb0, ncol) in zip(chunks, chunk_cols):
            sb_chunks.setdefault(sbi, []).append((lo, hi, q, cb0, ncol))

        buf_of = {s: i % NBUF for i, s in enumerate(proc_order)}

        def issue_gathers(sbi):
            msgs = msgs_bufs[buf_of[sbi]]
            if "gather" in abl:
                return
            t0 = sb_tile_base[sbi]
            for (lo, hi, q, cb0, ncol) in sb_chunks[sbi]:
                nrow = (hi - lo) * P
                if cb0 < HEADC:
                    idx_ap = gidxh[:, cb0:cb0 + ncol]
                else:
                    idx_ap = gidxt[:, cb0 - HEADC:cb0 - HEADC + ncol]
                nc.gpsimd.dma_gather(
                    msgs[:, (lo - t0) * P:(hi - t0) * P]
                    .rearrange("p (t e) -> p t e", e=D),
                    tab_ds[sbi % NPHASE][:, :],
                    idx_ap,
                    nrow, nrow, D, queue_num=q)

        for _g in range(NBUF - 1):
            issue_gathers(proc_order[_g])

        maskt = sb.tile([P, MASKC], f16)
        nc.sync.dma_start(out=maskt[:], in_=masks_d[:])
        wmT = sb.tile([P, D], f16)
        nc.sync.dma_start(out=wmT[:], in_=wmT_d[:])
        wsT = sb.tile([P, D], f16)
        nc.sync.dma_start(out=wsT[:], in_=wsT_d[:])

        # bias16 = (bm + bs) as f16 row
        nc.vector.tensor_tensor(out=bmt[:], in0=bmt[:], in1=bst[:], op=Op.add)
        bias16 = sb.tile([1, D], f16)
        nc.vector.tensor_copy(out=bias16[:], in_=bmt[:])
        ones1 = sb.tile([1, P], f16)
        nc.vector.memset(ones1[:], 1.0)
        zrow = sb.tile([1, SBW], f16)
        nc.vector.memset(zrow[:], 0.0)

        featsT = sb.tile([P, NLOC_PAD], f16)
        nc.sync.dma_start(out=featsT[:], in_=featsT_d[:])

        # wE[p,t] = (prod>0) * rsqrt(max(prod,1)), prod = degS*degD.
        # Computed in 2-superblock column chunks, each immediately followed
        # by that pair's strip builds, so sb0's strips are ready in a few us
        # and compute never starves behind a monolithic reciprocal.
        msk = sb.tile([P, TILES], f32, tag="mskP")
        wE = sb.tile([P, TILES], f32)
        wst_bufs = []
        for s in range(NSB):
            wt = sb.tile([P, wst_cols[s]], f16, tag=f"wst{s}", name=f"w{s}")
            wst_bufs.append(wt)

        def w_chunk(c0, c1):
            s_ = slice(c0, c1)
            nc.vector.tensor_scalar(out=msk[:, s_], in0=prod[:, s_],
                                    scalar1=0.0, scalar2=None, op0=Op.is_gt)
            nc.vector.tensor_scalar(out=prod[:, s_], in0=prod[:, s_],
                                    scalar1=1.0, scalar2=None, op0=Op.max)
            nc.vector.reciprocal(out=wE[:, s_], in_=prod[:, s_])
            nc.scalar.activation(out=wE[:, s_], in_=wE[:, s_], func=Act.Sqrt)
            nc.vector.tensor_tensor(out=wE[:, s_], in0=wE[:, s_],
                                    in1=msk[:, s_], op=Op.mult)

        def build_strips(sbi):
            if "scale" in abl:
                return
            t0 = sb_tile_base[sbi]
            wt = wst_bufs[sbi]
            for t in range(sb_tiles[sbi]):
                T0 = t0 + t
                mc, W, bc = tile_info[T0]
                wc = tile_wc[T0]
                nc.vector.tensor_scalar(
                    out=wt[:, wc:wc + W], in0=maskt[:, mc:mc + W],
                    scalar1=wE[:, T0:T0 + 1], scalar2=None, op0=Op.mult)

        # wE per superblock in PROCESSING order; first two processed
        # superblocks' strips built immediately so compute starts early
        for pi, s in enumerate(proc_order):
            w_chunk(sb_tile_base[s], sb_tile_base[s] + sb_tiles[s])
            if pi == 1:
                build_strips(proc_order[0])
                build_strips(proc_order[1])

        # ---------------- main loop ----------------
        def tiles_of(sbi):
            msgs = msgs_bufs[buf_of[sbi]]
            t0 = sb_tile_base[sbi]
            tsb = sb_tiles[sbi]
            bankA = psag.tile([P, SBW], f32, tag="aggA", space="PSUM")
            bankB = psag.tile([P, SBW], f32, tag="aggB", space="PSUM")
            if "aggmm" not in abl:
                nc.tensor.matmul(out=bankA[:], lhsT=zrow[:, :P],
                                 rhs=zrow[:], start=True, stop=False)
                nc.tensor.matmul(out=bankB[:], lhsT=zrow[:, :P],
                                 rhs=zrow[:], start=True, stop=False)
            for t in range(tsb):
                T0 = t0 + t
                if "aggmm" in abl:
                    continue
                mc, W, bc = tile_info[T0]
                if "scale" not in abl:
                    wc = tile_wc[T0]
                    rhs = wst_bufs[sbi][:, wc:wc + W]
                else:
                    rhs = maskt[:, mc:mc + W]
                bank = bankA if t % 2 == 0 else bankB
                nc.tensor.matmul(
                    out=bank[:, bc:bc + W],
                    lhsT=msgs[:, t * P:(t + 1) * P],
                    rhs=rhs,
                    start=False, stop=(t >= tsb - 2))
            return bankA, bankB

        def flush_of(sbi, bankA, bankB):
            if "aggmm" in abl or "flush" in abl:
                return
            rstT = srst.tile([P, SBW], f16, tag="rstT")
            nc.scalar.copy(out=rstT[:], in_=bankA[:])
            nc.vector.tensor_tensor(out=rstT[:], in0=bankB[:],
                                    in1=rstT[:], op=Op.add)
            # one [128,512] lin psum tile, column-sliced per 128-dst block;
            # single staging copy + single output DMA per superblock
            pmk = pslin.tile([P, SBW], f32, tag="pmk", space="PSUM")
            for b in range(4):
                pb = pmk[:, b * P:(b + 1) * P]
                nc.tensor.matmul(out=pb,
                                 lhsT=rstT[:, b * P:(b + 1) * P],
                                 rhs=wmT[:], start=True, stop=False)
                nc.tensor.matmul(out=pb,
                                 lhsT=featsT[:, (sbi * 4 + b) * P:
                                             (sbi * 4 + b + 1) * P],
                                 rhs=wsT[:], start=False, stop=False)
                nc.tensor.matmul(out=pb, lhsT=ones1[:], rhs=bias16[:],
                                 start=False, stop=True)
            stage = sstg.tile([P, SBW], f32, tag="stage")
            nc.scalar.copy(out=stage[:], in_=pmk[:])
            nc.sync.dma_start(
                out=out_d[sbi * SBW:(sbi + 1) * SBW, :]
                .rearrange("(b p) c -> p b c", p=P),
                in_=stage[:].rearrange("p (b c) -> p b c", c=D))

        def body():
            # flush deferred one superblock (emitted after the next tile
            # stream): PE then runs agg(N) while ACT copies banks(N-1), so
            # the flush matmuls never wait on the ACT round trip
            pending = None
            for pi in range(NSB):
                sbi = proc_order[pi]
                if pi + NBUF - 1 < NSB:
                    issue_gathers(proc_order[pi + NBUF - 1])
                if pi + 2 < NSB:
                    build_strips(proc_order[pi + 2])
                if pending is not None:
                    flush_of(*pending)
                banks = tiles_of(sbi)
                pending = (sbi, *banks)
            flush_of(*pending)

        body()


# --------------------------------------------------------------- entry point

def _build_program(geom, cfg):
    import concourse.bacc as bacc
    import concourse.mybir as mybir
    import concourse.tile as tile

    f32 = mybir.dt.float32
    f16 = mybir.dt.float16
    i16 = mybir.dt.int16

    nc = bacc.Bacc("TRN2", target_bir_lowering=False, debug=False,
                   enable_asserts=False, num_devices=NCORES,
                   num_swdge_queues=4)

    def inp(name, shape, dt):
        return nc.dram_tensor(name, shape, dt, kind="ExternalInput").ap()

    gidx = inp("gidx", [P, geom["GIDXC"]], i16)
    gprod = inp("gprod", [P, geom["TILES"]], f32)
    featsT = inp("featsT", [P, NLOC_PAD], f16)
    masks = inp("masks", [P, geom["MASKC"]], f16)
    wmT = inp("wmT", [P, D], f16)
    wsT = inp("wsT", [P, D], f16)
    bm = inp("bm", [1, D], f32)
    bs = inp("bs", [1, D], f32)
    tabs = [inp(f"tab{ph}", [geom["TABR"], D], f16) for ph in range(NPHASE)]
    out = nc.dram_tensor("out", [NLOC_PAD, D], f32, kind="ExternalOutput").ap()

    with tile.TileContext(nc) as tc:
        device_kernel(
            tc, [out],
            [gidx, gprod, featsT, masks, wmT, wsT, bm, bs, tabs],
            geom, cfg)
    nc.compile()
    return nc


LAST_EXEC_NS = None


def kernel(feats, src, dst, linear_skip_weight, linear_skip_bias,
           linear_msg_weight, linear_msg_bias):
    global LAST_EXEC_NS
    import os

    from concourse.bass_utils import run_bass_kernel_spmd

    feats = np.asarray(feats)
    in_maps, geom, order = _prep(
        feats, src, dst, linear_msg_weight, linear_msg_bias,
        linear_skip_weight, linear_skip_bias)
    abl = tuple(a for a in os.environ.get("GCN_ABL", "").split(",") if a)
    nc = _build_program(geom, cfg={"ABL": abl} if abl else {})
    trace = bool(int(os.environ.get("GCN_TRACE", "0")))
    res = run_bass_kernel_spmd(nc, in_maps, core_ids=list(range(NCORES)),
                               trace=trace)
    LAST_EXEC_NS = res.exec_time_ns
    if res.instructions_and_trace is not None:
        print("trace:", res.instructions_and_trace[1])
    out = np.empty((N, D), np.float32)
    rr = np.arange(NLOC)
    for k in range(NCORES):
        out[order[rr * NCORES + k]] = res.results[k]["out"][:NLOC]
    return out
